# revision 22
# baseline (speedup 1.0000x reference)
"""AlphaBorderPadding on 8 TRN2 NeuronCores.

Sharding: H rows across 8 cores, 512 owned + 8-row ghost zones per side; each
core runs all `offset` box-filter iterations locally (no collectives).  The
528-row slab is processed as 5 overlapping 128-row partition tiles, each
SBUF-resident in fp16 through all iterations.

State tiles are [128, W+2] fp16 with zero guard columns at 0 and W+1, so the
TensorE 3x3 box filter can read column-shifted rhs APs without bounds issues:
box3 = band-matmul (vertical, contraction along partitions) x 3 PSUM-accumulated
matmuls with rhs shifted -1/0/+1 (horizontal).  The mask channel always uses
this full-PE box (mask weights stay exact integers in PSUM f32; Ln/Sign read
PSUM directly).  RGB channels either do the same (RGB_PE3=1) or use a single
vertical matmul + ScalarE PSUM->SBUF copy + one VectorE tensor_tensor_scan
(telescoping 3-tap sum) per channel.

Per iteration: rq = Exp(-Ln(mw+eps)) (Reciprocal is banned); mask' = Sign(mw);
q = rq*mask'; qn = (m-1)*q; rgb' = rgb - box3(rgb)*qn.  All exactly 0 where
the reference divides 0/eps, and exact where mask==1.
"""

import os
import sys

import numpy as np

for _p in ("/opt/trn_rl_repo", "/root/.axon_site/_ro/trn_rl_repo"):
    if os.path.isdir(_p) and _p not in sys.path:
        sys.path.insert(0, _p)

H = W = 4096
NCORES = 8
OWN = H // NCORES            # 512 rows owned per core
HALO = 8                     # one halo row per iteration
SHARD = OWN + 2 * HALO       # 528 rows per core slab
TILE_STARTS = [0, 104, 216, 328, 400]          # tile row offsets in the slab
TILE_OUT = [                                   # (slab rows written, partitions read)
    ((0, 112), (0, 112)),
    ((112, 224), (8, 120)),
    ((224, 336), (8, 120)),
    ((336, 448), (8, 120)),
    ((448, 528), (48, 128)),
]
EPS = 1e-3
RGB_PE3 = os.environ.get("RGB_PE3", "1") == "1"

_cache = {}


def _build(iters: int, rgb_pe3: bool = RGB_PE3):
    from contextlib import ExitStack

    import concourse.bass as bass
    import concourse.tile as tile
    from concourse import bacc, mybir

    f32 = mybir.dt.float32
    f16 = mybir.dt.float16
    AF = mybir.ActivationFunctionType
    ALU = mybir.AluOpType

    nc = bacc.Bacc("TRN2", target_bir_lowering=False, debug=False,
                   num_devices=NCORES)

    alpha_d = nc.dram_tensor("alpha_s", [SHARD, W], f32, kind="ExternalInput").ap()
    rgb_d = nc.dram_tensor("rgb_s", [3, SHARD, W], f32, kind="ExternalInput").ap()
    band_d = nc.dram_tensor("band", [128, 128], f16, kind="ExternalInput").ap()
    out_d = nc.dram_tensor("out", [3, SHARD, W], f32, kind="ExternalOutput").ap()

    WG = W + 4                     # guarded state width; data cols [2, W+2)
    DS = slice(2, W + 2)           # data slice (4B-aligned for fp16 2x mode)

    with tile.TileContext(nc) as tc, ExitStack() as ctx:
        const = ctx.enter_context(tc.tile_pool(name="const", bufs=1))
        stg = ctx.enter_context(tc.tile_pool(name="stg", bufs=2))
        stm = ctx.enter_context(tc.tile_pool(name="stm", bufs=2))
        stc = ctx.enter_context(tc.tile_pool(name="stc", bufs=3))
        stn = ctx.enter_context(tc.tile_pool(name="stn", bufs=2))
        stb = ctx.enter_context(tc.tile_pool(name="stb", bufs=4))
        vb = ctx.enter_context(tc.tile_pool(name="vb", bufs=2))
        sm1 = ctx.enter_context(tc.tile_pool(name="sm1", bufs=1))
        sm2 = ctx.enter_context(tc.tile_pool(name="sm2", bufs=1))
        ob = ctx.enter_context(tc.tile_pool(name="ob", bufs=1))
        psum = ctx.enter_context(
            tc.tile_pool(name="psum", bufs=2, space=bass.MemorySpace.PSUM))

        band = const.tile([128, 128], f16)
        nc.sync.dma_start(band[:], band_d[:])
        eps_ap = const.tile([128, 1], f32)
        nc.vector.memset(eps_ap[:], EPS)
        zero_ap = const.tile([128, 1], f32)
        nc.vector.memset(zero_ap[:], 0.0)

        def state_tile(pool):
            t = pool.tile([128, WG], f16)
            nc.vector.memset(t[:, 0:2], 0.0)
            nc.vector.memset(t[:, W + 2:W + 4], 0.0)
            return t

        def box3_pe(src, half, acc):
            """3x3 box sum of guarded-state src for data cols
            [half*2048, (half+1)*2048) into psum acc (f32, accumulate)."""
            for j in range(4):
                b = half * 2048 + j * 512      # data col of block start
                for s in range(3):             # rhs shifted -1, 0, +1
                    nc.tensor.matmul(acc[:, j * 512:(j + 1) * 512],
                                     band[:], src[:, b + 1 + s: b + 1 + s + 512],
                                     start=(s == 0), stop=(s == 2))

        def box3_scan(src, dst):
            """box3 via vertical matmul + ACT copy + one DVE scan."""
            vbuf = vb.tile([128, W + 3], f16)
            nc.vector.memset(vbuf[:, 0:2], 0.0)
            nc.vector.memset(vbuf[:, W + 2:W + 3], 0.0)
            for h in range(2):
                acc = psum.tile([128, 2048], f32)
                for j in range(4):
                    b = h * 2048 + j * 512
                    nc.tensor.matmul(acc[:, j * 512:(j + 1) * 512], band[:],
                                     src[:, b + 2: b + 514])
                nc.scalar.copy(vbuf[:, 2 + h * 2048: 2 + (h + 1) * 2048], acc[:])
            nc.vector.tensor_tensor_scan(
                dst[:], vbuf[:, 3:W + 3], vbuf[:, 0:W],
                initial=vbuf[:, 2:3], op0=ALU.add, op1=ALU.subtract)

        for t, r0 in enumerate(TILE_STARTS):
            # --- load + init ---------------------------------------------
            m = None
            chans = []
            for ch in range(4):
                s = stg.tile([128, W], f32)
                if ch == 0:
                    nc.sync.dma_start(s[:], alpha_d[r0:r0 + 128, :])
                    m = state_tile(stm)
                    nc.vector.tensor_scalar(m[:, DS], s[:], 0.0, None,
                                            ALU.is_gt)
                else:
                    nc.sync.dma_start(s[:], rgb_d[ch - 1, r0:r0 + 128, :])
                    sh = sm2.tile([128, W], f16, name="cvt")
                    nc.scalar.copy(sh[:], s[:])
                    cc = state_tile(stc)
                    nc.vector.tensor_tensor(cc[:, DS], sh[:], m[:, DS],
                                            ALU.mult)
                    chans.append(cc)

            # --- iterate --------------------------------------------------
            for _ in range(iters):
                # mask channel: full box on PE; Ln/Sign straight from PSUM
                mnew = state_tile(stn)
                lnb = sm1.tile([128, W], f16)
                for h in range(2):
                    acc = psum.tile([128, 2048], f32)
                    box3_pe(m, h, acc)
                    nc.scalar.activation(lnb[:, h * 2048:(h + 1) * 2048],
                                         acc[:], AF.Ln, bias=eps_ap[:])
                    nc.scalar.activation(mnew[:, 2 + h * 2048:2 + (h + 1) * 2048],
                                         acc[:], AF.Sign, bias=zero_ap[:])
                rq = sm1.tile([128, W], f16)
                nc.scalar.activation(rq[:], lnb[:], AF.Exp, scale=-1.0)
                nm1 = sm1.tile([128, W], f16)
                nc.vector.tensor_scalar(nm1[:], m[:, DS], -1.0, None, ALU.add)
                qn = sm1.tile([128, W], f16)
                if rgb_pe3:
                    # PE box sums are exact zeros where mw==0, so no Sign
                    # gate is needed: qn = (m-1)/(mw+eps)
                    nc.vector.tensor_tensor(qn[:], nm1[:], rq[:], ALU.mult)
                else:
                    # scan residue can leak ~1e-4 into box where mw==0; gate
                    # by the (exact) dilated mask
                    q = sm1.tile([128, W], f16)
                    nc.vector.tensor_tensor(q[:], rq[:], mnew[:, DS], ALU.mult)
                    nc.vector.tensor_tensor(qn[:], nm1[:], q[:], ALU.mult)

                for c in range(3):
                    bord = state_tile(stb)
                    if rgb_pe3:
                        box = (sm2.tile([128, W], f16, name="boxc")
                               if c == 2 else None)
                        for h in range(2):
                            acc = psum.tile([128, 2048], f32)
                            box3_pe(chans[c], h, acc)
                            hs = slice(2 + h * 2048, 2 + (h + 1) * 2048)
                            hq = slice(h * 2048, (h + 1) * 2048)
                            if c == 2:
                                # balance: route one channel through ScalarE
                                # (PSUM->SBUF copy) so the multiply runs at
                                # DVE 2x instead of the 1x PSUM-read rate
                                nc.scalar.copy(box[:, hq], acc[:])
                                nc.vector.tensor_tensor(
                                    bord[:, hs], box[:, hq], qn[:, hq],
                                    ALU.mult)
                            else:
                                nc.vector.tensor_tensor(
                                    bord[:, hs], acc[:], qn[:, hq], ALU.mult)
                    else:
                        box = sm2.tile([128, W], f16)
                        box3_scan(chans[c], box)
                        nc.vector.tensor_tensor(bord[:, DS], box[:],
                                                qn[:], ALU.mult)
                    nc.vector.tensor_tensor(bord[:, DS], chans[c][:, DS],
                                            bord[:, DS], ALU.subtract)
                    chans[c] = bord
                m = mnew

            # --- clip + store --------------------------------------------
            (w0, w1), (p0, p1) = TILE_OUT[t]
            for c in range(3):
                o = ob.tile([128, W], f32)
                nc.scalar.activation(o[:], chans[c][:, DS], AF.Relu,
                                     bias=zero_ap[:])
                nc.vector.tensor_scalar(o[:], o[:], 1.0, None, ALU.min)
                nc.sync.dma_start(out_d[c, w0:w1, :], o[p0:p1, :])

    nc.compile()
    return nc


def _band_np():
    b = np.zeros((128, 128), dtype=np.float16)
    for k in range(128):
        for d in (-1, 0, 1):
            if 0 <= k + d < 128:
                b[k, k + d] = 1.0
    return b


def kernel(rgb, alpha, offset):
    from concourse.bass_utils import run_bass_kernel_spmd

    iters = int(offset)
    rgb = np.asarray(rgb, dtype=np.float32)
    alpha = np.asarray(alpha, dtype=np.float32)

    if iters not in _cache:
        _cache[iters] = _build(iters)
    nc = _cache[iters]

    band = _band_np()
    starts = [min(max(512 * k - HALO, 0), H - SHARD) for k in range(NCORES)]
    in_maps = []
    for k in range(NCORES):
        s = starts[k]
        in_maps.append({
            "alpha_s": np.ascontiguousarray(alpha[0, s:s + SHARD, :]),
            "rgb_s": np.ascontiguousarray(rgb[:, s:s + SHARD, :]),
            "band": band,
        })

    res = run_bass_kernel_spmd(nc, in_maps, core_ids=list(range(NCORES)))
    out = np.empty((3, H, W), dtype=np.float32)
    for k in range(NCORES):
        o = 512 * k - starts[k]
        out[:, 512 * k:512 * (k + 1), :] = res.results[k]["out"][:, o:o + 512, :]
    return out


# revision 28
# speedup vs baseline: 1.0161x; 1.0161x over previous
"""AlphaBorderPadding on 8 TRN2 NeuronCores.

Sharding: H rows across 8 cores, 512 owned + 8-row ghost zones per side; each
core runs all `offset` box-filter iterations locally (no collectives).  The
528-row slab is processed as 5 overlapping 128-row partition tiles, each
SBUF-resident in fp16 through all iterations.

State tiles are [128, W+2] fp16 with zero guard columns at 0 and W+1, so the
TensorE 3x3 box filter can read column-shifted rhs APs without bounds issues:
box3 = band-matmul (vertical, contraction along partitions) x 3 PSUM-accumulated
matmuls with rhs shifted -1/0/+1 (horizontal).  The mask channel always uses
this full-PE box (mask weights stay exact integers in PSUM f32; Ln/Sign read
PSUM directly).  RGB channels either do the same (RGB_PE3=1) or use a single
vertical matmul + ScalarE PSUM->SBUF copy + one VectorE tensor_tensor_scan
(telescoping 3-tap sum) per channel.

Per iteration: rq = Exp(-Ln(mw+eps)) (Reciprocal is banned); mask' = Sign(mw);
q = rq*mask'; qn = (m-1)*q; rgb' = rgb - box3(rgb)*qn.  All exactly 0 where
the reference divides 0/eps, and exact where mask==1.
"""

import os
import sys

import numpy as np

for _p in ("/opt/trn_rl_repo", "/root/.axon_site/_ro/trn_rl_repo"):
    if os.path.isdir(_p) and _p not in sys.path:
        sys.path.insert(0, _p)

H = W = 4096
NCORES = 8
OWN = H // NCORES            # 512 rows owned per core
HALO = 8                     # one halo row per iteration
SHARD = OWN + 2 * HALO       # 528 rows per core slab
TILE_STARTS = [0, 104, 216, 328, 400]          # tile row offsets in the slab
TILE_OUT = [                                   # (slab rows written, partitions read)
    ((0, 112), (0, 112)),
    ((112, 224), (8, 120)),
    ((224, 336), (8, 120)),
    ((336, 448), (8, 120)),
    ((448, 528), (48, 128)),
]
EPS = 1e-3
RGB_PE3 = os.environ.get("RGB_PE3", "1") == "1"

_cache = {}


def _build(iters: int, rgb_pe3: bool = RGB_PE3):
    from contextlib import ExitStack

    import concourse.bass as bass
    import concourse.tile as tile
    from concourse import bacc, mybir

    f32 = mybir.dt.float32
    f16 = mybir.dt.float16
    AF = mybir.ActivationFunctionType
    ALU = mybir.AluOpType

    nc = bacc.Bacc("TRN2", target_bir_lowering=False, debug=False,
                   num_devices=NCORES)

    alpha_d = nc.dram_tensor("alpha_s", [SHARD, W], f32, kind="ExternalInput").ap()
    rgb_d = nc.dram_tensor("rgb_s", [3, SHARD, W], f32, kind="ExternalInput").ap()
    band_d = nc.dram_tensor("band", [128, 128], f16, kind="ExternalInput").ap()
    out_d = nc.dram_tensor("out", [3, SHARD, W], f32, kind="ExternalOutput").ap()

    WG = W + 4                     # guarded state width; data cols [2, W+2)
    DS = slice(2, W + 2)           # data slice (4B-aligned for fp16 2x mode)

    with tile.TileContext(nc) as tc, ExitStack() as ctx:
        const = ctx.enter_context(tc.tile_pool(name="const", bufs=1))
        stg = ctx.enter_context(tc.tile_pool(name="stg", bufs=2))
        stm = ctx.enter_context(tc.tile_pool(name="stm", bufs=2))
        stc = ctx.enter_context(tc.tile_pool(name="stc", bufs=3))
        stn = ctx.enter_context(tc.tile_pool(name="stn", bufs=2))
        stb = ctx.enter_context(tc.tile_pool(name="stb", bufs=5 if rgb_pe3 else 4))
        vb = ctx.enter_context(tc.tile_pool(name="vb", bufs=2))
        sm1 = ctx.enter_context(tc.tile_pool(name="sm1", bufs=1))
        sm2 = ctx.enter_context(tc.tile_pool(name="sm2", bufs=1))
        ob = ctx.enter_context(tc.tile_pool(name="ob", bufs=1))
        psum = ctx.enter_context(
            tc.tile_pool(name="psum", bufs=2, space=bass.MemorySpace.PSUM))

        band = const.tile([128, 128], f16)
        nc.sync.dma_start(band[:], band_d[:])
        eps_ap = const.tile([128, 1], f32)
        nc.vector.memset(eps_ap[:], EPS)
        zero_ap = const.tile([128, 1], f32)
        nc.vector.memset(zero_ap[:], 0.0)

        def state_tile(pool):
            t = pool.tile([128, WG], f16)
            nc.vector.memset(t[:, 0:2], 0.0)
            nc.vector.memset(t[:, W + 2:W + 4], 0.0)
            return t

        def box3_pe(src, half, acc):
            """3x3 box sum of guarded-state src for data cols
            [half*2048, (half+1)*2048) into psum acc (f32, accumulate)."""
            for j in range(4):
                b = half * 2048 + j * 512      # data col of block start
                for s in range(3):             # rhs shifted -1, 0, +1
                    nc.tensor.matmul(acc[:, j * 512:(j + 1) * 512],
                                     band[:], src[:, b + 1 + s: b + 1 + s + 512],
                                     start=(s == 0), stop=(s == 2))

        def box3_scan(src, dst):
            """box3 via vertical matmul + ACT copy + one DVE scan."""
            vbuf = vb.tile([128, W + 3], f16)
            nc.vector.memset(vbuf[:, 0:2], 0.0)
            nc.vector.memset(vbuf[:, W + 2:W + 3], 0.0)
            for h in range(2):
                acc = psum.tile([128, 2048], f32)
                for j in range(4):
                    b = h * 2048 + j * 512
                    nc.tensor.matmul(acc[:, j * 512:(j + 1) * 512], band[:],
                                     src[:, b + 2: b + 514])
                nc.scalar.copy(vbuf[:, 2 + h * 2048: 2 + (h + 1) * 2048], acc[:])
            nc.vector.tensor_tensor_scan(
                dst[:], vbuf[:, 3:W + 3], vbuf[:, 0:W],
                initial=vbuf[:, 2:3], op0=ALU.add, op1=ALU.subtract)

        for t, r0 in enumerate(TILE_STARTS):
            # --- load + init ---------------------------------------------
            m = None
            chans = []
            for ch in range(4):
                s = stg.tile([128, W], f32)
                if ch == 0:
                    nc.sync.dma_start(s[:], alpha_d[r0:r0 + 128, :])
                    m = state_tile(stm)
                    nc.vector.tensor_scalar(m[:, DS], s[:], 0.0, None,
                                            ALU.is_gt)
                else:
                    nc.sync.dma_start(s[:], rgb_d[ch - 1, r0:r0 + 128, :])
                    cc = state_tile(stc)
                    if rgb_pe3:
                        sh = sm2.tile([128, W], f16, name="cvt")
                        nc.scalar.copy(sh[:], s[:])
                        nc.vector.tensor_tensor(cc[:, DS], sh[:], m[:, DS],
                                                ALU.mult)
                    else:
                        nc.vector.tensor_tensor(cc[:, DS], s[:], m[:, DS],
                                                ALU.mult)
                    chans.append(cc)

            # --- iterate --------------------------------------------------
            for _ in range(iters):
                # mask channel: full box on PE; Ln/Sign straight from PSUM
                mnew = state_tile(stn)
                lnb = sm1.tile([128, W], f16)
                for h in range(2):
                    acc = psum.tile([128, 2048], f32)
                    box3_pe(m, h, acc)
                    nc.scalar.activation(lnb[:, h * 2048:(h + 1) * 2048],
                                         acc[:], AF.Ln, bias=eps_ap[:])
                    nc.scalar.activation(mnew[:, 2 + h * 2048:2 + (h + 1) * 2048],
                                         acc[:], AF.Sign, bias=zero_ap[:])
                rq = sm1.tile([128, W], f16)
                nc.scalar.activation(rq[:], lnb[:], AF.Exp, scale=-1.0)
                nm1 = sm1.tile([128, W], f16)
                nc.vector.tensor_scalar(nm1[:], m[:, DS], -1.0, None, ALU.add)
                qn = sm1.tile([128, W], f16)
                if rgb_pe3:
                    # PE box sums are exact zeros where mw==0, so no Sign
                    # gate is needed: qn = (m-1)/(mw+eps)
                    nc.vector.tensor_tensor(qn[:], nm1[:], rq[:], ALU.mult)
                else:
                    # scan residue can leak ~1e-4 into box where mw==0; gate
                    # by the (exact) dilated mask
                    q = sm1.tile([128, W], f16)
                    nc.vector.tensor_tensor(q[:], rq[:], mnew[:, DS], ALU.mult)
                    nc.vector.tensor_tensor(qn[:], nm1[:], q[:], ALU.mult)

                for c in range(3):
                    bord = state_tile(stb)
                    if rgb_pe3:
                        box = (sm2.tile([128, W], f16, name="boxc")
                               if c == 2 else None)
                        for h in range(2):
                            acc = psum.tile([128, 2048], f32)
                            box3_pe(chans[c], h, acc)
                            hs = slice(2 + h * 2048, 2 + (h + 1) * 2048)
                            hq = slice(h * 2048, (h + 1) * 2048)
                            if c == 2:
                                # balance: route one channel through ScalarE
                                # (PSUM->SBUF copy) so the multiply runs at
                                # DVE 2x instead of the 1x PSUM-read rate
                                nc.scalar.copy(box[:, hq], acc[:])
                                nc.vector.tensor_tensor(
                                    bord[:, hs], box[:, hq], qn[:, hq],
                                    ALU.mult)
                            else:
                                nc.vector.tensor_tensor(
                                    bord[:, hs], acc[:], qn[:, hq], ALU.mult)
                    else:
                        box = sm2.tile([128, W], f16)
                        box3_scan(chans[c], box)
                        nc.vector.tensor_tensor(bord[:, DS], box[:],
                                                qn[:], ALU.mult)
                    nc.vector.tensor_tensor(bord[:, DS], chans[c][:, DS],
                                            bord[:, DS], ALU.subtract)
                    chans[c] = bord
                m = mnew

            # --- clip + store --------------------------------------------
            (w0, w1), (p0, p1) = TILE_OUT[t]
            for c in range(3):
                o = ob.tile([128, W], f32)
                nc.vector.tensor_scalar(o[:], chans[c][:, DS], 0.0, 1.0,
                                        ALU.max, ALU.min)
                nc.sync.dma_start(out_d[c, w0:w1, :], o[p0:p1, :])

    nc.compile()
    return nc


def _band_np():
    b = np.zeros((128, 128), dtype=np.float16)
    for k in range(128):
        for d in (-1, 0, 1):
            if 0 <= k + d < 128:
                b[k, k + d] = 1.0
    return b


def kernel(rgb, alpha, offset):
    from concourse.bass_utils import run_bass_kernel_spmd

    iters = int(offset)
    rgb = np.asarray(rgb, dtype=np.float32)
    alpha = np.asarray(alpha, dtype=np.float32)

    if iters not in _cache:
        _cache[iters] = _build(iters)
    nc = _cache[iters]

    band = _band_np()
    starts = [min(max(512 * k - HALO, 0), H - SHARD) for k in range(NCORES)]
    in_maps = []
    for k in range(NCORES):
        s = starts[k]
        in_maps.append({
            "alpha_s": np.ascontiguousarray(alpha[0, s:s + SHARD, :]),
            "rgb_s": np.ascontiguousarray(rgb[:, s:s + SHARD, :]),
            "band": band,
        })

    res = run_bass_kernel_spmd(nc, in_maps, core_ids=list(range(NCORES)))
    out = np.empty((3, H, W), dtype=np.float32)
    for k in range(NCORES):
        o = 512 * k - starts[k]
        out[:, 512 * k:512 * (k + 1), :] = res.results[k]["out"][:, o:o + 512, :]
    return out
